# revision 1
# baseline (speedup 1.0000x reference)
"""GroupContrastLoss on 8 trn2 NeuronCores via Bass/Tile.

Math (reference):
  pos   = (gt == 1)                              [B,K,H,W]
  fnorm = feat / max(||feat||_C, eps)            per-pixel L2 over C
  k0    = einsum('bkhw,bchw->kc', pos, fnorm)    [K,C]   (global sum!)
  k0n   = k0 / max(||k0||_C, eps)
  logits= einsum('kc,bchw->bkhw', k0n, fnorm)/tau
  loss  = -sum(pos * log_softmax(logits, k)) / sum(pos)

Sharding: pixels (b, hw) split into 8 contiguous shards (2 per batch
image, 32768 pixels each). Each core computes a partial k0 [19,256]
(AllReduce on-device between the two phases), plus scalar partials
(sum pos*logp, sum pos) combined on host.

Feat is read from HBM exactly once, via gpsimd casting DMAs (f32 in
DRAM -> bf16 in SBUF, cast in flight) into a persistent tile-major
stash [128, NT, 2, 2048] that phase 2 reuses for the logit matmuls.
Pixel-major feat/gt come from 2-byte xbar DMA transposes; each
transpose instruction costs ~1.7-3.6us serialized on its issuing
engine, so tiles are processed in PAIRS and each pair uses one big
feat transpose ([128,8192] -> [128,64,128]) and one gt transpose,
cutting the transpose instruction count 3x. Per-pixel inv-norms come
from chunked fused square+accumulate ops (scalar_tensor_tensor on DVE
-- NB its cousin tensor_tensor_reduce hangs TRN2 hw -- and
Square+accum_out on Scalar); invr folds into the transposed gt mask
(posw) for the k0 accumulation (16 bf16 matmuls per tile, PE otherwise
free in phase 1).

Phase 2 computes logits as [K,512] bf16 matmuls (k0n^T stationary,
itself produced by a tiny xbar transpose, no PSUM), stages PSUM->SBUF
as bf16 K-major, one xbar transpose per pair back to pixel-major, and
a pair-batched masked softmax. Ln and the loss combination are
deferred to one batched tail because EXP and LN live in different hw
activation tables (per-tile Ln thrashes ~2.6us of table loads).
"""

import numpy as np

TAU = 0.07
EPS = 1e-12
B, C, H, W, K = 4, 256, 256, 256, 19
HW = H * W
NCORES = 8
SHARD = B * HW // NCORES        # 32768 pixels per core
TILE_PIX = 2048                 # pixels per tile iteration
NCH = TILE_PIX // 128           # 16 chunks of 128 pixels
NT = SHARD // TILE_PIX          # 16 tile iterations
NP = NT // 2                    # 8 tile pairs
CH = C // 2                     # 128, feat channel half
KP = 32                         # classes padded to 32 for DMA transpose
GP = 512                        # pixels per logit matmul group
NG = TILE_PIX // GP             # 4 groups per tile

_CACHE = {}


def _build_nc(ncores=NCORES):
    import concourse.bass as bass
    import concourse.bass_isa as bass_isa
    import concourse.bacc as bacc
    import concourse.mybir as mybir
    from concourse import tile, masks

    f32 = mybir.dt.float32
    bf16 = mybir.dt.bfloat16
    AX = mybir.AxisListType
    AF = mybir.ActivationFunctionType
    ALU = mybir.AluOpType

    nc = bacc.Bacc("TRN2", target_bir_lowering=False, debug=False,
                   num_devices=ncores)

    feat_in = nc.dram_tensor("feat_s", [C, SHARD], f32, kind="ExternalInput")
    gt_in = nc.dram_tensor("gt_s", [K, SHARD], f32, kind="ExternalInput")
    out_part = nc.dram_tensor("part", [2, 1], f32, kind="ExternalOutput")

    with tile.TileContext(nc) as tc:
        with (
            tc.tile_pool(name="persist", bufs=1) as pp,
            tc.tile_pool(name="ft", bufs=2) as pft,
            tc.tile_pool(name="small", bufs=2) as ps,
            tc.tile_pool(name="p2", bufs=1) as p2,
            tc.tile_pool(name="dram", bufs=1, space="DRAM") as pd,
        ):
            # persistent stashes
            fa16 = pp.tile([128, NT, 2, TILE_PIX], bf16)  # feat, tile-major
            posT16 = pp.tile([128, NT * NCH, KP], bf16)   # gt transposed
            invr_all = pp.tile([128, NT * NCH], f32)
            npos_cols = pp.tile([128, NT * NCH], f32)
            s_all = pp.tile([128, NT * NCH], f32)
            araw_all = pp.tile([128, NT * NCH], f32)
            sq_v = pp.tile([128, C], bf16)                # SoS scratch outs
            sq_s = pp.tile([128, C], bf16)

            # pair-sized K-major staging, shared by phase 1 (gt) and
            # phase 2 (z); rows K:KP zeroed once, never rewritten
            NGT = 2
            gt16 = [pp.tile([KP, 2 * TILE_PIX], bf16, name=f"gt16_{i}")
                    for i in range(NGT)]
            nc.vector.memset(gt16[0][:], 0.0)
            nc.gpsimd.memset(gt16[1][:], 0.0)

            # ---------------- phase 1: k0 accumulation ----------------
            with tc.tile_pool(name="psA", bufs=1, space="PSUM") as psA:
                k0_ps = psA.tile([K, C], f32)

                def _cast_pair(pr):
                    psl = slice(2 * pr * TILE_PIX, (2 * pr + 2) * TILE_PIX)
                    nc.gpsimd.dma_start(gt16[pr % NGT][0:K, :],
                                        gt_in[:, psl])
                    nc.gpsimd.dma_start(fa16[:, 2 * pr:2 * pr + 2, 0, :],
                                        feat_in[0:CH, psl])
                    nc.gpsimd.dma_start(fa16[:, 2 * pr:2 * pr + 2, 1, :],
                                        feat_in[CH:C, psl])

                def _transposes(pr):
                    ptsl = slice(2 * pr * NCH, (2 * pr + 2) * NCH)
                    ftp2 = pft.tile([128, 2, 2, NCH, 128], bf16, tag="ftp",
                                    name=f"ftp_{pr}")
                    nc.sync.dma_start(ftp2[:],
                                      fa16[:, 2 * pr:2 * pr + 2, :, :],
                                      transpose=True)
                    nc.sync.dma_start(posT16[:, ptsl, :], gt16[pr % NGT][:],
                                      transpose=True)
                    return ftp2

                for pr in range(2):
                    _cast_pair(pr)
                ftps = {0: _transposes(0)}
                NDV = 9   # SoS chunks on DVE; rest on Scalar
                for pr in range(NP):
                    if pr + 2 < NP:
                        _cast_pair(pr + 2)
                    if pr + 1 < NP:
                        ftps[pr + 1] = _transposes(pr + 1)
                    ftp2 = ftps.pop(pr)
                    for tt in range(2):
                        t = 2 * pr + tt
                        tsl = slice(t * NCH, (t + 1) * NCH)
                        ss = ps.tile([128, NCH], f32, tag="ss",
                                     name=f"ss_{t}")
                        for j in range(NCH):
                            src = ftp2[:, tt, :, j, :]
                            if j < NDV:
                                nc.vector.scalar_tensor_tensor(
                                    out=sq_v[:], in0=src, scalar=1.0,
                                    in1=src, op0=ALU.mult, op1=ALU.mult,
                                    accum_out=ss[:, j:j + 1])
                            else:
                                nc.scalar.activation(
                                    sq_s[:], src, AF.Square,
                                    accum_out=ss[:, j:j + 1])
                        srt = ps.tile([128, NCH], f32, tag="srt",
                                      name=f"srt_{t}")
                        nc.scalar.sqrt(srt[:], ss[:])
                        nc.vector.reciprocal(invr_all[:, tsl], srt[:])

                        posw = ps.tile([128, NCH, K], bf16, tag="posw",
                                       name=f"posw_{t}")
                        nc.vector.tensor_mul(
                            posw[:], posT16[:, tsl, 0:K],
                            invr_all[:, tsl].unsqueeze(2).broadcast_to(
                                [128, NCH, K]))
                        nc.vector.tensor_reduce(
                            npos_cols[:, tsl], posT16[:, tsl, 0:K],
                            axis=AX.X, op=ALU.add)

                        for j in range(NCH):
                            nc.tensor.matmul(
                                k0_ps[:], posw[:, j, :],
                                ftp2[:, tt, :, j, :],
                                start=(t == 0 and j == 0),
                                stop=(t == NT - 1 and j == NCH - 1),
                                skip_group_check=True)

                k0_sb = pp.tile([K, C], f32)
                nc.scalar.copy(k0_sb[:], k0_ps[:])

            # ---------------- AllReduce k0 across 8 cores ----------------
            k0_loc = pd.tile([K, C], f32)
            k0_sum = pd.tile([K, C], f32)
            nc.sync.dma_start(k0_loc[:], k0_sb[:])
            nc.gpsimd.collective_compute(
                "AllReduce", ALU.add,
                ins=[k0_loc.opt()],
                outs=[k0_sum.opt()],
                replica_groups=[list(range(ncores))],
            )
            k0t = pp.tile([K, C], f32)
            nc.sync.dma_start(k0t[:], k0_sum[:])

            # k0ns = (k0 / max(||k0||, eps)) / tau, transposed to [c, 2, K]
            k0sq = pp.tile([K, C], f32)
            ssk = pp.tile([K, 1], f32)
            nc.scalar.activation(k0sq[:], k0t[:], AF.Square, accum_out=ssk[:])
            sk = pp.tile([K, 1], f32)
            nc.scalar.sqrt(sk[:], ssk[:])
            skm = pp.tile([K, 1], f32)
            nc.vector.tensor_scalar_max(skm[:], sk[:], EPS)
            invk = pp.tile([K, 1], f32)
            nc.vector.reciprocal(invk[:], skm[:])
            invks = pp.tile([K, 1], f32)
            nc.scalar.mul(invks[:], invk[:], 1.0 / TAU)
            # bf16 k0ns staged in the zero-padded 32-row tile, one tiny xbar
            # transpose gives k0n^T [c, 2, K] without touching PE/PSUM
            nc.vector.tensor_scalar_mul(gt16[0][0:K, 0:C], k0t[:], invks[:])
            k0nT16 = pp.tile([128, 2, KP], bf16)
            nc.sync.dma_start(k0nT16[:], gt16[0][:, 0:C], transpose=True)

            # ---------------- phase 2: logits, log-softmax, loss ----------
            with tc.tile_pool(name="psB", bufs=2, space="PSUM") as psB:
                for t in range(NT):
                    pr, tt = t // 2, t % 2
                    # two 2-bank PSUM tiles per tile; halves accumulated
                    lgA = psB.tile([K, 2, GP], f32, tag="lgA",
                                   name=f"lgA_{t}")
                    lgB = psB.tile([K, 2, GP], f32, tag="lgB",
                                   name=f"lgB_{t}")
                    lgs = [lgA[:, 0, :], lgA[:, 1, :], lgB[:, 0, :],
                           lgB[:, 1, :]]
                    for g in range(NG):
                        gsl = slice(g * GP, (g + 1) * GP)
                        nc.tensor.matmul(
                            lgs[g], k0nT16[:, 0, 0:K], fa16[:, t, 0, gsl],
                            start=True, stop=False, skip_group_check=True)
                    for g in range(NG):
                        gsl = slice(g * GP, (g + 1) * GP)
                        nc.tensor.matmul(
                            lgs[g], k0nT16[:, 1, 0:K], fa16[:, t, 1, gsl],
                            start=False, stop=True, skip_group_check=True)

                    # PSUM -> K-major bf16 staging (per tile), one xbar
                    # transpose + batched softmax per pair
                    zs = gt16[pr % NGT]
                    zo = tt * TILE_PIX
                    nc.scalar.copy(zs[0:K, zo:zo + 2 * GP], lgA[:])
                    nc.vector.tensor_copy(zs[0:K, zo + 2 * GP:zo + 4 * GP],
                                          lgB[:])
                    if tt == 0:
                        continue
                    ptsl = slice(2 * pr * NCH, (2 * pr + 2) * NCH)
                    zT = p2.tile([128, 2 * NCH, KP], bf16, tag="zT",
                                 name=f"zT_{pr}")
                    nc.sync.dma_start(zT[:], zs[:], transpose=True)

                    ib = invr_all[:, ptsl].unsqueeze(2).broadcast_to(
                        [128, 2 * NCH, K])
                    y = p2.tile([128, 2 * NCH, K], f32, tag="y",
                                name=f"y_{pr}")
                    nc.gpsimd.tensor_mul(y[:], zT[:, :, 0:K], ib)
                    nc.scalar.activation(y[:], y[:], AF.Exp)
                    nc.vector.reduce_sum(s_all[:, ptsl], y[:], axis=AX.X)
                    # pos*z, reduced over classes (zT overwritten in place)
                    nc.vector.tensor_mul(zT[:, :, 0:K], zT[:, :, 0:K],
                                         posT16[:, ptsl, 0:K])
                    nc.vector.reduce_sum(araw_all[:, ptsl], zT[:, :, 0:K],
                                         axis=AX.X)

                # deferred loss tail, batched over all 256 columns (keeps Ln
                # out of the loop: EXP and LN live in different activation
                # tables on hw, so per-tile Ln thrashes the table loads)
                nc.scalar.activation(s_all[:], s_all[:], AF.Ln)
                nc.vector.tensor_mul(araw_all[:], araw_all[:], invr_all[:])
                nc.gpsimd.tensor_mul(npos_cols[:], npos_cols[:], s_all[:])
                nc.vector.tensor_sub(araw_all[:], araw_all[:],
                                     npos_cols[:])

                # final partials: [sum pos*logp, sum pos] -- npos was
                # consumed above, so re-reduce num_pos from posT16
                lred = pp.tile([128, 2], f32)
                nc.vector.reduce_sum(
                    lred[:, 0:1], araw_all[:], axis=AX.X)
                nc.vector.tensor_reduce(
                    lred[:, 1:2], posT16[:, :, 0:K], axis=AX.XY, op=ALU.add)
                lfin = pp.tile([128, 2], f32)
                nc.gpsimd.partition_all_reduce(
                    lfin[:], lred[:], channels=128,
                    reduce_op=bass_isa.ReduceOp.add)
                nc.sync.dma_start(out_part[:], lfin[0:1, 0:2])

    nc.compile()
    return nc


def kernel(feat: np.ndarray, gt: np.ndarray) -> np.ndarray:
    from concourse.bass_utils import run_bass_kernel_spmd

    if "nc" not in _CACHE:
        _CACHE["nc"] = _build_nc()
    nc = _CACHE["nc"]

    feat_r = np.ascontiguousarray(feat, dtype=np.float32).reshape(B, C, HW)
    gt_r = np.ascontiguousarray(gt, dtype=np.float32).reshape(B, K, HW)
    per_batch = NCORES // B                       # 2 shards per image
    span = HW // per_batch                        # 32768
    in_maps = []
    for m in range(NCORES):
        b, lo = m // per_batch, (m % per_batch) * span
        in_maps.append({
            "feat_s": np.ascontiguousarray(feat_r[b, :, lo:lo + span]),
            "gt_s": np.ascontiguousarray(gt_r[b, :, lo:lo + span]),
        })

    res = run_bass_kernel_spmd(nc, in_maps, list(range(NCORES)))
    _CACHE["last_results"] = res
    parts = np.stack([r["part"].reshape(2) for r in res.results])
    loss_sum = float(np.sum(parts[:, 0].astype(np.float64)))
    num_pos = float(np.sum(parts[:, 1].astype(np.float64)))
    return np.asarray(-loss_sum / num_pos, dtype=np.float32)

